# revision 29
# baseline (speedup 1.0000x reference)
"""PointPillar loss on 8 Trainium2 NeuronCores.

Data-parallel over the batch dim (B=8 -> one batch element per core).
Each core gathers the ~1150 elements of loc/clf that the loss actually
touches (one dma_gather of 256B rows + an on-chip one-hot select),
computes its partial smooth-L1 / focal sums on-device, and the host sums
the 8 partial scalars.

Self-contained: hardcodes the problem shapes from the spec.
"""

import sys

import numpy as np

if "/opt/trn_rl_repo" not in sys.path:
    sys.path.insert(0, "/opt/trn_rl_repo")

B, A, H, W = 8, 2, 496, 432
N_BOXES, N_BG = 50, 1000
PLANE = H * W  # 214272
N_CORES = 8
COLS = 9
N_SLOTS = 128 * COLS  # 1152 slots; 1150 used
CHUNK = 64            # dma_gather row size in f32 elements (256B)
N_ROWS = 4 * PLANE // CHUNK  # 13392
ALPHA = 0.25
BETA_LOC = 2.0

# smalls[128, 96] column layout (f32)
REM0, REM1 = 0, 9      # element position within gathered row
G0, G1 = 9, 11         # gt-box coordinate pairs
INVDA = 11             # 1/sqrt(anchor_w^2 + anchor_h^2)
WF0, WF1 = 12, 21      # focal weights (0 on smooth-L1/pad slots)
WS0, WS1 = 21, 30      # smooth-L1 weights (0 elsewhere)
C0, C1 = 30, 32        # coefficients turning gt pairs into x_gt / y_gt
IO0, IO1 = 32, 96      # iota 0..63
SMALL_COLS = 96

_CACHE = {}


def _grid(flat):
    """Map a length-1152 slot vector to the on-chip [128, 9] layout.

    Slot n lives at partition n % 128, free column n // 128 (dma_gather's
    native output order) — so slots 0..99 (the smooth-L1 entries) occupy
    column 0, one per partition, letting the gt target act as a
    per-partition scalar operand.
    """
    return np.ascontiguousarray(flat.reshape(COLS, 128).T)


def _const_cols():
    wf = np.zeros(N_SLOTS, np.float32)
    wf[100:150] = -ALPHA / ((B - 1) * (N_BOXES - 1))
    wf[150:1150] = -ALPHA / ((B - 1) * (N_BG - 1))
    ws = np.zeros(N_SLOTS, np.float32)
    ws[0:100] = 0.5 * BETA_LOC / (B * N_BOXES)
    c = np.zeros((128, 2), np.float32)
    c[0:50] = (0.5, 0.5)    # x_gt = 0.5*c0 + 0.5*c2
    c[50:100] = (1.5, -0.5)  # y_gt = 1.5*c1 - 0.5*c3
    return _grid(wf), _grid(ws), c


_WF2D, _WS2D, _C2D = _const_cols()


def build_bass(skip_par=False, skip_act=False, dump=None):
    import concourse.bacc as bacc
    import concourse.bass as bass
    import concourse.mybir as mybir
    from concourse import bass_isa
    from concourse.library_config import mlp

    f32 = mybir.dt.float32
    i16 = mybir.dt.int16
    op = mybir.AluOpType
    act = mybir.ActivationFunctionType

    nc = bacc.Bacc("TRN2", target_bir_lowering=False, debug=False,
                   num_devices=N_CORES)
    planes = nc.dram_tensor("planes", [N_ROWS, CHUNK], f32, kind="ExternalInput")
    smalls = nc.dram_tensor("smalls", [128, SMALL_COLS], f32, kind="ExternalInput")
    idxs = nc.dram_tensor("idxs", [128, N_SLOTS // 16], i16, kind="ExternalInput")
    out_shape = [1, 1] if dump is None else [128, 2 * COLS]
    outp = nc.dram_tensor("out", out_shape, f32, kind="ExternalOutput")

    from contextlib import ExitStack

    with ExitStack() as ctx:
        block = ctx.enter_context(nc.Block())

        def sb(name, shape, dt=f32):
            return ctx.enter_context(nc.sbuf_tensor(name, shape, dt))

        sm = sb("sm", [128, SMALL_COLS])
        idxb = sb("idxb", [128, N_SLOTS // 16], i16)
        v64 = sb("v64", [128, COLS, CHUNK])
        mask3 = sb("mask3", [128, COLS, CHUNK])
        vm = sb("vm", [128, COLS, CHUNK])
        v = sb("v", [128, COLS])
        tg = sb("tg", [128, 2])
        junk2 = sb("junk2", [128, 2])
        t = sb("t", [128, COLS])
        neg = sb("neg", [128, COLS])
        ab = sb("ab", [128, COLS])
        mm1 = sb("mm1", [128, COLS])
        q = sb("q", [128, COLS])
        r = sb("r", [128, COLS])
        s = sb("s", [128, COLS])
        pcl = sb("pcl", [128, COLS])
        lnb = sb("lnb", [128, COLS])
        cb = sb("cb", [128, COLS])
        c2b = sb("c2b", [128, COLS])
        fo = sb("fo", [128, COLS])
        junk9a = sb("junk9a", [128, COLS])
        junk9b = sb("junk9b", [128, COLS])
        junk9c = sb("junk9c", [128, COLS])
        acc1 = sb("acc1", [128, 1])
        acc2 = sb("acc2", [128, 1])
        accf = sb("accf", [128, 1])
        pr = sb("pr", [128, 1])
        warm = sb("warm", [1, 1])
        io = ctx.enter_context(nc.semaphore("io"))
        gs = ctx.enter_context(nc.semaphore("gs"))
        dve_p = ctx.enter_context(nc.semaphore("dve_p"))
        act_done = ctx.enter_context(nc.semaphore("act_done"))
        dve_done = ctx.enter_context(nc.semaphore("dve_done"))
        od = ctx.enter_context(nc.semaphore("od"))
        dve_c = ctx.enter_context(nc.semaphore("dve_c"))

        @block.sync
        def _(sync: bass.BassEngine):
            sync.dma_start(out=sm[:], in_=smalls[:]).then_inc(io, 16)
            sync.dma_start(out=idxb[:], in_=idxs[:]).then_inc(io, 16)

        @block.gpsimd
        def _(g: bass.BassGpSimd):
            g.load_library(mlp)
            g.wait_ge(io, 32)
            # single_packet=False: 1152 idxs -> 73 descriptors per lane, far
            # beyond the 64-descriptor/16KB single-packet limit.
            g.dma_gather(
                v64[:], planes[:], idxb[:], N_SLOTS, N_SLOTS, CHUNK,
                single_packet=False,
            ).then_inc(gs, 16)
            g.wait_ge(dve_done, 1)
            if dump is not None:
                dump_tiles = {"v": v, "t": t, "pcl": pcl, "lnb": lnb, "s": s,
                              "c2b": c2b, "fo": fo, "mask_sum": vm,
                              "ab": ab, "mm1": mm1, "q": q, "r": r}
                dt_ = dump_tiles[dump]
                g.dma_start(out=outp[:, 0:COLS], in_=dt_[:]).then_inc(od, 16)
                g.dma_start(out=outp[:, COLS:2 * COLS], in_=t[:]).then_inc(od, 16)
            elif skip_par:
                g.dma_start(out=outp[:], in_=acc2[0:1, 0:1]).then_inc(od, 16)
            else:
                g.partition_all_reduce(
                    pr[:], acc2[:], channels=128, reduce_op=bass_isa.ReduceOp.add
                )
                g.dma_start(out=outp[:], in_=pr[0:1, 0:1]).then_inc(od, 16)
            g.wait_ge(od, 32 if dump is not None else 16)

        @block.vector
        def _(d: bass.BassVectorEngine):
            # Every DVE op incs dve_c at completion; each dependent op first
            # waits for its producers' counts. Same-engine program order alone
            # does NOT make writes visible on this HW (Tile does the same).
            cnt = [0]

            def step(ins):
                ins.then_inc(dve_c, 1)
                cnt[0] += 1
                return cnt[0]

            def need(k):
                d.wait_ge(dve_c, k)

            d.wait_ge(io, 32)
            # Tg = sum_j G[:, j] * C[:, j]  (per-partition gt target)
            step(d.tensor_tensor(
                out=junk2[:], in0=sm[:, G0:G1], in1=sm[:, C0:C1], op=op.mult
            ))
            need(cnt[0])
            k_tg = step(d.tensor_reduce(
                out=tg[:, 0:1], in_=junk2[:], axis=mybir.AxisListType.X, op=op.add
            ))
            # one-hot mask: mask3[p, i, j] = (iota[j] == rem[p, i])
            for i in range(COLS):
                step(d.tensor_scalar(
                    out=mask3[:, i, :], in0=sm[:, IO0:IO1],
                    scalar1=sm[:, REM0 + i:REM0 + i + 1], scalar2=None,
                    op0=op.is_equal,
                ))
            d.wait_ge(gs, 16)
            need(cnt[0])  # all masks written
            step(d.tensor_tensor(out=vm[:], in0=v64[:], in1=mask3[:], op=op.mult))
            need(cnt[0])
            k_v = step(d.tensor_reduce(
                out=v[:], in_=vm[:], axis=mybir.AxisListType.X, op=op.add
            ))
            need(k_v)
            # t = (v - Tg) / da
            step(d.tensor_scalar(
                out=junk9c[:], in0=v[:], scalar1=tg[:, 0:1], scalar2=None,
                op0=op.subtract,
            ))
            need(cnt[0])
            k_t = step(d.tensor_scalar(
                out=t[:], in0=junk9c[:], scalar1=sm[:, INVDA:INVDA + 1],
                scalar2=None, op0=op.mult,
            ))
            pcl_ins = d.tensor_scalar(
                out=pcl[:], in0=v[:], scalar1=1e-12, scalar2=None, op0=op.max
            )
            if skip_act:
                step(pcl_ins)  # fo reads pcl directly in this mode
            else:
                pcl_ins.then_inc(dve_p, 1)  # ACT waits on dve_p instead
            need(k_t)
            # huber*2 = t^2 - (max(|t|,1) - 1)^2
            step(d.tensor_scalar(
                out=neg[:], in0=t[:], scalar1=-1.0, scalar2=None, op0=op.mult
            ))
            need(cnt[0])
            step(d.tensor_tensor(out=ab[:], in0=t[:], in1=neg[:], op=op.max))
            need(cnt[0])
            step(d.tensor_scalar(
                out=mm1[:], in0=ab[:], scalar1=1.0, scalar2=-1.0,
                op0=op.max, op1=op.add,
            ))
            step(d.tensor_tensor(out=q[:], in0=t[:], in1=t[:], op=op.mult))
            need(cnt[0])
            step(d.tensor_tensor(out=r[:], in0=mm1[:], in1=mm1[:], op=op.mult))
            need(cnt[0])
            step(d.tensor_tensor(out=s[:], in0=q[:], in1=r[:], op=op.subtract))
            # focal: fo = (1-v)^2 * ln(max(v, eps))
            step(d.tensor_scalar(
                out=cb[:], in0=v[:], scalar1=-1.0, scalar2=1.0,
                op0=op.mult, op1=op.add,
            ))
            need(cnt[0])
            step(d.tensor_tensor(out=c2b[:], in0=cb[:], in1=cb[:], op=op.mult))
            need(cnt[0])
            # weighted partial sums (plain mult+reduce; TTR crashes this HW)
            step(d.tensor_tensor(out=junk9a[:], in0=s[:], in1=sm[:, WS0:WS1],
                                 op=op.mult))
            need(cnt[0])
            step(d.tensor_reduce(
                out=acc1[:], in_=junk9a[:], axis=mybir.AxisListType.X, op=op.add
            ))
            if not skip_act:
                d.wait_ge(act_done, 1)
            step(d.tensor_tensor(
                out=fo[:], in0=c2b[:], in1=pcl[:] if skip_act else lnb[:],
                op=op.mult,
            ))
            need(cnt[0])
            step(d.tensor_tensor(out=junk9b[:], in0=fo[:], in1=sm[:, WF0:WF1],
                                 op=op.mult))
            need(cnt[0])
            step(d.tensor_reduce(
                out=accf[:], in_=junk9b[:], axis=mybir.AxisListType.X, op=op.add
            ))
            need(cnt[0])
            d.tensor_tensor(
                out=acc2[:], in0=accf[:], in1=acc1[:], op=op.add
            ).then_inc(dve_done, 1)

        if not skip_act:
            @block.scalar
            def _(sc: bass.BassScalarEngine):
                sc.wait_ge(io, 32)
                # warm the Ln table while the gather is in flight
                sc.activation(warm[:], sm[0:1, INVDA:INVDA + 1], act.Ln)
                sc.wait_ge(dve_p, 1)
                sc.activation(lnb[:], pcl[:], act.Ln).then_inc(act_done, 1)

    nc.compile()
    return nc


def host_inputs(regression_targets, classification_targets, gt_boxes, loc, clf,
                anchor):
    reg = np.asarray(regression_targets).astype(np.int64)
    cls_t = np.asarray(classification_targets).astype(np.int64)
    gt = np.asarray(gt_boxes, dtype=np.float32)
    loc = np.asarray(loc, dtype=np.float32)
    clf = np.asarray(clf, dtype=np.float32)
    anc = np.asarray(anchor, dtype=np.float32)
    inv_da = np.float32(1.0) / np.sqrt(anc[0] * anc[0] + anc[1] * anc[1],
                                       dtype=np.float32)

    iota = np.arange(CHUNK, dtype=np.float32)

    in_maps = []
    for b in range(B):
        planes_b = np.ascontiguousarray(
            np.stack([loc[b, 0, 0], loc[b, 0, 1], clf[b, 0, 1], clf[b, 0, 0]])
        ).reshape(N_ROWS, CHUNK)
        y, x = reg[b, :, 1], reg[b, :, 0]
        base = y * W + x
        flat = np.zeros(N_SLOTS, np.int64)
        flat[0:50] = 0 * PLANE + base
        flat[50:100] = 1 * PLANE + base
        flat[100:150] = 2 * PLANE + base
        flat[150:1150] = 3 * PLANE + cls_t[b, :, 2] * W + cls_t[b, :, 1]

        # dma_gather index layout: index n sits at partition n % 16,
        # column n // 16, replicated across the 8 groups of 16 partitions.
        rows16 = np.ascontiguousarray(
            (flat // CHUNK).astype(np.int16).reshape(N_SLOTS // 16, 16).T
        )
        idx16 = np.tile(rows16, (8, 1))  # [128, 72]

        smalls_b = np.zeros((128, SMALL_COLS), np.float32)
        smalls_b[:, REM0:REM1] = _grid((flat % CHUNK).astype(np.float32))
        smalls_b[0:50, G0:G1] = gt[b][:, [0, 2]]
        smalls_b[50:100, G0:G1] = gt[b][:, [1, 3]]
        smalls_b[:, INVDA] = inv_da
        smalls_b[:, WF0:WF1] = _WF2D
        smalls_b[:, WS0:WS1] = _WS2D
        smalls_b[:, C0:C1] = _C2D
        smalls_b[:, IO0:IO1] = iota
        in_maps.append({"planes": planes_b, "smalls": smalls_b, "idxs": idx16})
    return in_maps


def run(in_maps, trace=False):
    from concourse.bass_utils import run_bass_kernel_spmd

    if "nc" not in _CACHE:
        _CACHE["nc"] = build_bass()
    res = run_bass_kernel_spmd(
        _CACHE["nc"], in_maps, core_ids=list(range(N_CORES)), trace=trace
    )
    return res


def kernel(regression_targets, classification_targets, gt_boxes, loc, size,
           clf, occupancy, angle, heading, anchor):
    in_maps = host_inputs(regression_targets, classification_targets, gt_boxes,
                          loc, clf, anchor)
    res = run(in_maps)
    total = np.float32(0.0)
    for r in res.results:
        total += np.float32(r["out"][0, 0])
    return np.array(total, dtype=np.float32)


# revision 32
# speedup vs baseline: 1.1215x; 1.1215x over previous
"""PointPillar loss on 8 Trainium2 NeuronCores.

Data-parallel over the batch dim (B=8 -> one batch element per core).
Each core gathers the ~1150 elements of loc/clf that the loss actually
touches (one dma_gather of 256B rows + an on-chip one-hot select),
computes its partial smooth-L1 / focal sums on-device, and the host sums
the 8 partial scalars.

Self-contained: hardcodes the problem shapes from the spec.
"""

import sys

import numpy as np

if "/opt/trn_rl_repo" not in sys.path:
    sys.path.insert(0, "/opt/trn_rl_repo")

B, A, H, W = 8, 2, 496, 432
N_BOXES, N_BG = 50, 1000
PLANE = H * W  # 214272
N_CORES = 8
COLS = 9
N_SLOTS = 128 * COLS  # 1152 slots; 1150 used
CHUNK = 64            # dma_gather row size in f32 elements (256B)
N_ROWS = 4 * PLANE // CHUNK  # 13392
ALPHA = 0.25
BETA_LOC = 2.0

# smalls[128, 132] column layout (f32 view)
IDX0, IDX1 = 0, 36     # dma_gather row indices, int16 bits ([128, 72] i16)
REM0, REM1 = 36, 45    # element position within gathered row
G0, G1 = 45, 47        # gt-box coordinate pairs
INVDA = 47             # 1/sqrt(anchor_w^2 + anchor_h^2)
WF0, WF1 = 48, 57      # focal weights (0 on smooth-L1/pad slots)
WS0, WS1 = 57, 66      # smooth-L1 weights (0 elsewhere)
C0, C1 = 66, 68        # coefficients turning gt pairs into x_gt / y_gt
IO0, IO1 = 68, 132     # iota 0..63
SMALL_COLS = 132

_CACHE = {}


def _grid(flat):
    """Map a length-1152 slot vector to the on-chip [128, 9] layout.

    Slot n lives at partition n % 128, free column n // 128 (dma_gather's
    native output order) — so slots 0..99 (the smooth-L1 entries) occupy
    column 0, one per partition, letting the gt target act as a
    per-partition scalar operand.
    """
    return np.ascontiguousarray(flat.reshape(COLS, 128).T)


def _const_cols():
    wf = np.zeros(N_SLOTS, np.float32)
    wf[100:150] = -ALPHA / ((B - 1) * (N_BOXES - 1))
    wf[150:1150] = -ALPHA / ((B - 1) * (N_BG - 1))
    ws = np.zeros(N_SLOTS, np.float32)
    ws[0:100] = 0.5 * BETA_LOC / (B * N_BOXES)
    c = np.zeros((128, 2), np.float32)
    c[0:50] = (0.5, 0.5)    # x_gt = 0.5*c0 + 0.5*c2
    c[50:100] = (1.5, -0.5)  # y_gt = 1.5*c1 - 0.5*c3
    return _grid(wf), _grid(ws), c


_WF2D, _WS2D, _C2D = _const_cols()


def build_bass(skip_par=False, skip_act=False, no_dve_sems=False,
               no_gather=False):
    import concourse.bacc as bacc
    import concourse.bass as bass
    import concourse.mybir as mybir
    from concourse import bass_isa
    from concourse.library_config import mlp
    from contextlib import ExitStack

    f32 = mybir.dt.float32
    i16 = mybir.dt.int16
    op = mybir.AluOpType
    act = mybir.ActivationFunctionType

    nc = bacc.Bacc("TRN2", target_bir_lowering=False, debug=False,
                   num_devices=N_CORES)
    planes = nc.dram_tensor("planes", [N_ROWS, CHUNK], f32, kind="ExternalInput")
    smalls = nc.dram_tensor("smalls", [128, SMALL_COLS], f32, kind="ExternalInput")
    outp = nc.dram_tensor("out", [1, 1], f32, kind="ExternalOutput")

    with ExitStack() as ctx:
        block = ctx.enter_context(nc.Block())

        def sb(name, shape, dt=f32):
            return ctx.enter_context(nc.sbuf_tensor(name, shape, dt))

        sm = sb("sm", [128, SMALL_COLS])
        v64 = sb("v64", [128, COLS, CHUNK])
        mask3 = sb("mask3", [128, COLS, CHUNK])
        vm = sb("vm", [128, COLS, CHUNK])
        v = sb("v", [128, COLS])
        tg = sb("tg", [128, 2])
        junk2 = sb("junk2", [128, 2])
        t = sb("t", [128, COLS])
        neg = sb("neg", [128, COLS])
        ab = sb("ab", [128, COLS])
        mm1 = sb("mm1", [128, COLS])
        q = sb("q", [128, COLS])
        r = sb("r", [128, COLS])
        s = sb("s", [128, COLS])
        pcl = sb("pcl", [128, COLS])
        lnb = sb("lnb", [128, COLS])
        cb = sb("cb", [128, COLS])
        c2b = sb("c2b", [128, COLS])
        fo = sb("fo", [128, COLS])
        j9a = sb("j9a", [128, COLS])
        j9b = sb("j9b", [128, COLS])
        tot = sb("tot", [128, COLS])
        acc2 = sb("acc2", [128, 1])
        pr = sb("pr", [128, 1])
        warm = sb("warm", [1, 1])
        io = ctx.enter_context(nc.semaphore("io"))
        gs = ctx.enter_context(nc.semaphore("gs"))
        dve_p = ctx.enter_context(nc.semaphore("dve_p"))
        act_done = ctx.enter_context(nc.semaphore("act_done"))
        dve_done = ctx.enter_context(nc.semaphore("dve_done"))
        par_done = ctx.enter_context(nc.semaphore("par_done"))
        od = ctx.enter_context(nc.semaphore("od"))
        dve_c = ctx.enter_context(nc.semaphore("dve_c"))

        @block.vector
        def _(d: bass.BassVectorEngine):
            # Every DVE op incs dve_c at completion; dependent ops wait for
            # their producers' counts. Same-engine program order alone does
            # NOT make writes visible on this HW (Tile does the same).
            cnt = [0]

            def step(ins):
                ins.then_inc(dve_c, 1)
                cnt[0] += 1
                return cnt[0]

            def need(k):
                if not no_dve_sems:
                    d.wait_ge(dve_c, k)

            d.wait_ge(io, 16)
            # Tg = sum_j G[:, j] * C[:, j]  (per-partition gt target)
            step(d.tensor_tensor(
                out=junk2[:], in0=sm[:, G0:G1], in1=sm[:, C0:C1], op=op.mult
            ))
            need(cnt[0])
            step(d.tensor_reduce(
                out=tg[:, 0:1], in_=junk2[:], axis=mybir.AxisListType.X, op=op.add
            ))
            # one-hot mask: mask3[p, i, j] = (iota[j] == rem[p, i])
            for i in range(COLS):
                step(d.tensor_scalar(
                    out=mask3[:, i, :], in0=sm[:, IO0:IO1],
                    scalar1=sm[:, REM0 + i:REM0 + i + 1], scalar2=None,
                    op0=op.is_equal,
                ))
            d.wait_ge(gs, 16)
            need(cnt[0])  # all masks written
            step(d.tensor_tensor(out=vm[:], in0=v64[:], in1=mask3[:], op=op.mult))
            need(cnt[0])
            step(d.tensor_reduce(
                out=v[:], in_=vm[:], axis=mybir.AxisListType.X, op=op.add
            ))
            need(cnt[0])  # v ready
            # ln input first so ACT starts ASAP (inc goes to dve_p, not dve_c)
            pcl_ins = d.tensor_scalar(
                out=pcl[:], in0=v[:], scalar1=1e-12, scalar2=None, op0=op.max
            )
            if skip_act:
                step(pcl_ins)
            else:
                pcl_ins.then_inc(dve_p, 1)
            k_pcl = cnt[0]
            cb_k = step(d.tensor_scalar(
                out=cb[:], in0=v[:], scalar1=-1.0, scalar2=1.0,
                op0=op.mult, op1=op.add,
            ))
            # t = (v - Tg) / da   (tg/inv settled long ago)
            t_k = step(d.tensor_scalar(
                out=t[:], in0=v[:], scalar1=tg[:, 0:1],
                scalar2=sm[:, INVDA:INVDA + 1], op0=op.subtract, op1=op.mult,
            ))
            need(cb_k)
            c2b_k = step(d.tensor_tensor(out=c2b[:], in0=cb[:], in1=cb[:],
                                         op=op.mult))
            need(t_k)
            # huber*2 = t^2 - (max(|t|,1) - 1)^2;  |t| = max(-t, t) fused
            ab_k = step(d.scalar_tensor_tensor(
                out=ab[:], in0=t[:], scalar=-1.0, in1=t[:],
                op0=op.mult, op1=op.max,
            ))
            step(d.tensor_tensor(out=q[:], in0=t[:], in1=t[:], op=op.mult))
            need(ab_k)
            mm1_k = step(d.tensor_scalar(
                out=mm1[:], in0=ab[:], scalar1=1.0, scalar2=-1.0,
                op0=op.max, op1=op.add,
            ))
            need(mm1_k)
            r_k = step(d.tensor_tensor(out=r[:], in0=mm1[:], in1=mm1[:],
                                       op=op.mult))
            need(r_k)  # q completed earlier; cumulative count covers it
            s_k = step(d.tensor_tensor(out=s[:], in0=q[:], in1=r[:],
                                       op=op.subtract))
            need(s_k)
            j9a_k = step(d.tensor_tensor(out=j9a[:], in0=s[:],
                                         in1=sm[:, WS0:WS1], op=op.mult))
            if not skip_act:
                d.wait_ge(act_done, 1)
            need(c2b_k)
            fo_k = step(d.tensor_tensor(
                out=fo[:], in0=c2b[:], in1=pcl[:] if skip_act else lnb[:],
                op=op.mult,
            ))
            need(fo_k)
            j9b_k = step(d.tensor_tensor(out=j9b[:], in0=fo[:],
                                         in1=sm[:, WF0:WF1], op=op.mult))
            need(j9b_k)  # covers j9a too
            # tot = j9a + j9b with fused per-partition accumulate
            d.scalar_tensor_tensor(
                out=tot[:], in0=j9a[:], scalar=1.0, in1=j9b[:],
                op0=op.mult, op1=op.add, accum_out=acc2[:],
            ).then_inc(dve_done, 1)
            _ = k_pcl

        @block.gpsimd
        def _(g: bass.BassGpSimd):
            g.load_library(mlp)
            g.wait_ge(io, 16)
            # single_packet=False: 1152 idxs -> 73 descriptors per lane, far
            # beyond the 64-descriptor/16KB single-packet limit.
            if no_gather:
                g.sem_inc(gs, 16)
            else:
                g.dma_gather(
                    v64[:], planes[:], sm[:, IDX0:IDX1].bitcast(i16),
                    N_SLOTS, N_SLOTS, CHUNK, single_packet=False,
                ).then_inc(gs, 16)
            g.wait_ge(dve_done, 1)
            if skip_par:
                g.memcpy(pr[0:1, 0:1], acc2[0:1, 0:1]).then_inc(par_done, 1)
            else:
                g.partition_all_reduce(
                    pr[:], acc2[:], channels=128,
                    reduce_op=bass_isa.ReduceOp.add,
                ).then_inc(par_done, 1)

        @block.sync
        def _(sync: bass.BassEngine):
            sync.dma_start(out=sm[:], in_=smalls[:]).then_inc(io, 16)
            sync.wait_ge(par_done, 1)
            sync.dma_start(out=outp[:], in_=pr[0:1, 0:1]).then_inc(od, 16)
            sync.wait_ge(od, 16)

        if not skip_act:
            @block.scalar
            def _(sc: bass.BassScalarEngine):
                # warm the Ln table immediately (const input, no DMA dep)
                sc.activation(warm[:], nc.const_aps.tensor(1.0, (1, 1)),
                              act.Ln)
                sc.wait_ge(dve_p, 1)
                sc.activation(lnb[:], pcl[:], act.Ln).then_inc(act_done, 1)

    nc.compile()
    return nc


def host_inputs(regression_targets, classification_targets, gt_boxes, loc, clf,
                anchor):
    reg = np.asarray(regression_targets).astype(np.int64)
    cls_t = np.asarray(classification_targets).astype(np.int64)
    gt = np.asarray(gt_boxes, dtype=np.float32)
    loc = np.asarray(loc, dtype=np.float32)
    clf = np.asarray(clf, dtype=np.float32)
    anc = np.asarray(anchor, dtype=np.float32)
    inv_da = np.float32(1.0) / np.sqrt(anc[0] * anc[0] + anc[1] * anc[1],
                                       dtype=np.float32)

    iota = np.arange(CHUNK, dtype=np.float32)

    in_maps = []
    for b in range(B):
        planes_b = np.ascontiguousarray(
            np.stack([loc[b, 0, 0], loc[b, 0, 1], clf[b, 0, 1], clf[b, 0, 0]])
        ).reshape(N_ROWS, CHUNK)
        y, x = reg[b, :, 1], reg[b, :, 0]
        base = y * W + x
        flat = np.zeros(N_SLOTS, np.int64)
        flat[0:50] = 0 * PLANE + base
        flat[50:100] = 1 * PLANE + base
        flat[100:150] = 2 * PLANE + base
        flat[150:1150] = 3 * PLANE + cls_t[b, :, 2] * W + cls_t[b, :, 1]

        # dma_gather index layout: index n sits at partition n % 16,
        # column n // 16, replicated across the 8 groups of 16 partitions.
        rows16 = np.ascontiguousarray(
            (flat // CHUNK).astype(np.int16).reshape(N_SLOTS // 16, 16).T
        )
        idx16 = np.tile(rows16, (8, 1))  # [128, 72]

        smalls_b = np.zeros((128, SMALL_COLS), np.float32)
        smalls_b[:, IDX0:IDX1] = idx16.view(np.float32)
        smalls_b[:, REM0:REM1] = _grid((flat % CHUNK).astype(np.float32))
        smalls_b[0:50, G0:G1] = gt[b][:, [0, 2]]
        smalls_b[50:100, G0:G1] = gt[b][:, [1, 3]]
        smalls_b[:, INVDA] = inv_da
        smalls_b[:, WF0:WF1] = _WF2D
        smalls_b[:, WS0:WS1] = _WS2D
        smalls_b[:, C0:C1] = _C2D
        smalls_b[:, IO0:IO1] = iota
        in_maps.append({"planes": planes_b, "smalls": smalls_b})
    return in_maps


def run(in_maps, trace=False):
    from concourse.bass_utils import run_bass_kernel_spmd

    if "nc" not in _CACHE:
        _CACHE["nc"] = build_bass()
    res = run_bass_kernel_spmd(
        _CACHE["nc"], in_maps, core_ids=list(range(N_CORES)), trace=trace
    )
    return res


def kernel(regression_targets, classification_targets, gt_boxes, loc, size,
           clf, occupancy, angle, heading, anchor):
    in_maps = host_inputs(regression_targets, classification_targets, gt_boxes,
                          loc, clf, anchor)
    res = run(in_maps)
    total = np.float32(0.0)
    for r in res.results:
        total += np.float32(r["out"][0, 0])
    return np.array(total, dtype=np.float32)
